# revision 7
# baseline (speedup 1.0000x reference)
"""DigitCaps dynamic-routing kernel for 8 TRN2 NeuronCores.

Strategy: shard the C=1152 input capsules across the 8 cores (144 each) and
keep the full batch B=256 on every core.  The routing iterations use the
factored form (never materializing u_hat = x @ W, which would be 189 MB):

  s[b,u,o]    = sum_{c,i} x[b,i,c] * (coef[c,u] * W[c,u,o,i])     (matmul, K=(c,i))
  v           = squash(s)
  G[ci,uo]    = sum_b x[b,i,c] * v[b,u,o]                          (matmul, K=b)
  agr[c,u]    = (1/B) * sum_{o,i} W[c,u,o,i] * G[(c,i),(u,o)]      (mult + selector matmul)
  b_logits   += agr ; coef = softmax_u(b_logits)                   (tiny, c-local)

Only cross-core traffic: AllGather of the per-core partial s ([256,160] f32)
once per routing iteration (4 total).  The c-sharded agreement/logits state is
fully core-local.  Iteration 1's uniform coef=0.1 is folded into a 0.1
pre-scale of the x operand used by the s-matmul (and cancelled for later
iterations by scaling the coefficient-expansion constant by 10).
"""

import sys

if "/opt/trn_rl_repo" not in sys.path:
    sys.path.insert(0, "/opt/trn_rl_repo")

import numpy as np

import concourse.bacc as bacc
import concourse.tile as tile
from concourse import mybir
from concourse.bass_utils import run_bass_kernel_spmd

F32 = mybir.dt.float32

B = 256          # batch
IU = 8           # in_unit (i)
C = 1152         # input capsules
U = 10           # output capsules
O = 16           # unit size
N_CORES = 8
CL = C // N_CORES          # 144 local capsules
CI = CL * IU               # 1152 local (c,i) rows
K = CI // 128              # 9 contraction tiles
UO = U * O                 # 160
NROUTE = 4


def _build_program():
    nc = bacc.Bacc(
        "TRN2",
        target_bir_lowering=False,
        debug=False,
        enable_asserts=False,
        num_devices=N_CORES,
    )

    xp_d = nc.dram_tensor("xp", [128, K * B], F32, kind="ExternalInput").ap()
    xb_d = nc.dram_tensor("xb", [128, 2 * CI], F32, kind="ExternalInput").ap()
    w1_d = nc.dram_tensor("w1", [128, K * UO], F32, kind="ExternalInput").ap()
    sel_d = nc.dram_tensor("sel", [128, 16], F32, kind="ExternalInput").ap()
    exp_d = nc.dram_tensor("exp16", [16, 128], F32, kind="ExternalInput").ap()
    out_d = nc.dram_tensor("out", [B, U, O, 1], F32, kind="ExternalOutput").ap()

    with tile.TileContext(nc) as tc:
        with (
            tc.tile_pool(name="persist", bufs=1) as pp,
            tc.tile_pool(name="work", bufs=2) as wp,
            tc.tile_pool(name="gsb", bufs=2) as gp,
            tc.tile_pool(name="pm", bufs=3) as pmp,
            tc.tile_pool(name="sps", bufs=2, space="PSUM") as sps,
            tc.tile_pool(name="gps", bufs=2, space="PSUM") as gps,
            tc.tile_pool(name="aps", bufs=2, space="PSUM") as aps,
            tc.tile_pool(name="cxps", bufs=1, space="PSUM") as cxps,
            tc.tile_pool(name="dram", bufs=2, space="DRAM") as dram,
        ):
            # ---- load persistent inputs ----
            xp_s = pp.tile([128, K * B], F32, tag="xp")
            xb_s = pp.tile([128, 2 * CI], F32, tag="xb")
            w1_s = pp.tile([128, K * UO], F32, tag="w1")
            sel_s = pp.tile([128, 16], F32, tag="sel")
            exp_s = pp.tile([16, 128], F32, tag="exp16")
            b_state = pp.tile([16, K * U], F32, tag="bstate")

            for k in range(K):
                nc.sync.dma_start(xp_s[:, k * B:(k + 1) * B], xp_d[:, k * B:(k + 1) * B])
                nc.sync.dma_start(w1_s[:, k * UO:(k + 1) * UO], w1_d[:, k * UO:(k + 1) * UO])
            nc.sync.dma_start(xb_s[:, :CI], xb_d[:, :CI])
            nc.sync.dma_start(xb_s[:, CI:], xb_d[:, CI:])
            nc.sync.dma_start(sel_s[:], sel_d)
            nc.sync.dma_start(exp_s[:], exp_d)

            weff = w1_s  # iteration 0 uses raw W (coef folded into xp scale)

            for r in range(NROUTE):
                # ---- s partial: [b,(u,o)] += xp^T @ weff, contraction (c,i) ----
                s_stage = wp.tile([128, 2 * UO], F32, tag="s_stage")
                for g in range(2):
                    s_ps = sps.tile([128, UO], F32, tag="s_ps")
                    for k in range(K):
                        nc.tensor.matmul(
                            s_ps[:],
                            lhsT=xp_s[:, k * B + g * 128: k * B + (g + 1) * 128],
                            rhs=weff[:, k * UO:(k + 1) * UO],
                            start=(k == 0),
                            stop=(k == K - 1),
                        )
                    nc.vector.tensor_copy(s_stage[:, g * UO:(g + 1) * UO], s_ps[:])

                # ---- AllGather partials, local 8-way sum ----
                cc_in = dram.tile([B, UO], F32, tag="cc_in")
                cc_out = dram.tile(
                    [N_CORES * B, UO], F32, tag="cc_out", addr_space="Shared"
                )
                nc.sync.dma_start(
                    cc_in.opt().rearrange("(g p) f -> p g f", g=2), s_stage[:]
                )
                nc.gpsimd.collective_compute(
                    "AllGather",
                    mybir.AluOpType.bypass,
                    replica_groups=[list(range(N_CORES))],
                    ins=[cc_in.opt()],
                    outs=[cc_out.opt()],
                )
                sg_s = wp.tile([128, N_CORES * 2 * UO], F32, tag="sg")
                nc.sync.dma_start(
                    sg_s[:], cc_out.opt().rearrange("(r g p) f -> p (r g) f", r=N_CORES, g=2)
                )
                s_sb = wp.tile([128, 2 * UO], F32, tag="s_sb")
                nc.vector.reduce_sum(
                    s_sb[:],
                    sg_s[:].rearrange("p (r f) -> p f r", r=N_CORES),
                    axis=mybir.AxisListType.X,
                )

                # ---- squash: v = s * sqrt(n2) / (1 + n2) ----
                sq = wp.tile([128, 2 * UO], F32, tag="sq")
                nc.vector.tensor_mul(sq[:], s_sb[:], s_sb[:])
                n2 = wp.tile([128, 2 * U], F32, tag="n2")
                nc.vector.reduce_sum(
                    n2[:], sq[:].rearrange("p (t u o) -> p (t u) o", t=2, u=U),
                    axis=mybir.AxisListType.X,
                )
                rt = wp.tile([128, 2 * U], F32, tag="rt")
                nc.scalar.sqrt(rt[:], n2[:])
                dn = wp.tile([128, 2 * U], F32, tag="dn")
                nc.vector.tensor_scalar_add(dn[:], n2[:], 1.0)
                rd = wp.tile([128, 2 * U], F32, tag="rd")
                nc.vector.reciprocal(rd[:], dn[:])
                f = wp.tile([128, 2 * U], F32, tag="f")
                nc.vector.tensor_mul(f[:], rt[:], rd[:])
                v_s = wp.tile([128, 2 * UO], F32, tag="v")
                nc.vector.tensor_mul(
                    v_s[:].rearrange("p (t u o) -> p t u o", t=2, u=U),
                    s_sb[:].rearrange("p (t u o) -> p t u o", t=2, u=U),
                    f[:].rearrange("p (t u) -> p t u", t=2).unsqueeze(3).broadcast_to((128, 2, U, O)),
                )

                if r == NROUTE - 1:
                    nc.sync.dma_start(
                        out_d.rearrange("(g p) u o one -> p g (u o one)", g=2), v_s[:]
                    )
                    break

                # ---- G[(c,i),(u,o)] = sum_b x * v ; agr[c,u] via sel matmul ----
                agr_sb = wp.tile([16, K * U], F32, tag="agr")
                g_sb = gp.tile([128, K * UO], F32, tag="g_sb")
                for m in range(K):
                    g_ps = gps.tile([128, UO], F32, tag="g_ps")
                    for t in range(2):
                        nc.tensor.matmul(
                            g_ps[:],
                            lhsT=xb_s[:, t * CI + m * 128: t * CI + (m + 1) * 128],
                            rhs=v_s[:, t * UO:(t + 1) * UO],
                            start=(t == 0),
                            stop=(t == 1),
                        )
                    pm = pmp.tile([128, UO], F32, tag="pmt")
                    nc.vector.tensor_mul(pm[:], g_ps[:], w1_s[:, m * UO:(m + 1) * UO])
                    a_ps = aps.tile([16, UO], F32, tag="a_ps")
                    nc.tensor.matmul(a_ps[:], lhsT=sel_s[:], rhs=pm[:], start=True, stop=True)
                    nc.vector.reduce_sum(
                        agr_sb[:, m * U:(m + 1) * U],
                        a_ps[:].rearrange("p (u o) -> p u o", u=U),
                        axis=mybir.AxisListType.X,
                    )

                # ---- logits update + softmax over u (c-local, tiny) ----
                if r == 0:
                    nc.vector.tensor_copy(b_state[:], agr_sb[:])
                else:
                    nc.vector.tensor_add(b_state[:], b_state[:], agr_sb[:])
                eb = wp.tile([16, K * U], F32, tag="eb")
                nc.scalar.activation(eb[:], b_state[:], mybir.ActivationFunctionType.Exp)
                den = wp.tile([16, K], F32, tag="den")
                nc.vector.reduce_sum(
                    den[:], eb[:].rearrange("p (k u) -> p k u", k=K),
                    axis=mybir.AxisListType.X,
                )
                rden = wp.tile([16, K], F32, tag="rden")
                nc.vector.reciprocal(rden[:], den[:])
                cnorm = wp.tile([16, K * U], F32, tag="cnorm")
                nc.vector.tensor_mul(
                    cnorm[:].rearrange("p (k u) -> p k u", k=K),
                    eb[:].rearrange("p (k u) -> p k u", k=K),
                    rden[:].rearrange("p k -> p k", k=K).unsqueeze(2).broadcast_to((16, K, U)),
                )

                # ---- expand coef to (c,i) partitions; W_eff = W * coef ----
                cx_ps = cxps.tile([128, K * U], F32, tag="cx")
                nc.tensor.matmul(cx_ps[:], lhsT=exp_s[:], rhs=cnorm[:], start=True, stop=True)
                cx_sb = wp.tile([128, K * U], F32, tag="cx_sb")
                nc.vector.tensor_copy(cx_sb[:], cx_ps[:])
                weff = wp.tile([128, K * UO], F32, tag="weff")
                nc.vector.tensor_mul(
                    weff[:].rearrange("p (k u o) -> p k u o", k=K, u=U),
                    w1_s[:].rearrange("p (k u o) -> p k u o", k=K, u=U),
                    cx_sb[:].rearrange("p (k u) -> p k u", k=K).unsqueeze(3).broadcast_to((128, K, U, O)),
                )

    nc.compile()
    return nc


_PROGRAM_CACHE = {}


def _get_program():
    if "nc" not in _PROGRAM_CACHE:
        _PROGRAM_CACHE["nc"] = _build_program()
    return _PROGRAM_CACHE["nc"]


def _make_in_maps(x, W):
    x = np.ascontiguousarray(x, dtype=np.float32)
    W = np.ascontiguousarray(W, dtype=np.float32)
    sel = np.zeros((128, 16), dtype=np.float32)
    for p in range(128):
        sel[p, p // IU] = 1.0 / B
    exp16 = np.zeros((16, 128), dtype=np.float32)
    for p in range(128):
        exp16[p // IU, p] = 10.0  # cancels the 0.1 pre-scale of xp

    in_maps = []
    for core in range(N_CORES):
        c0 = core * CL
        xc = x[:, :, c0:c0 + CL]                    # [B, I, CL]
        Wc = W[c0:c0 + CL]                          # [CL, U, O, I]
        # xp[p, k*B + b] = 0.1 * x[b, i, c], ci = k*128+p = c_rel*8+i
        xp = 0.1 * xc.transpose(2, 1, 0).reshape(CI, B)
        xp = np.ascontiguousarray(
            xp.reshape(K, 128, B).transpose(1, 0, 2).reshape(128, K * B)
        )
        # xb[p, t*CI + ci] = x[t*128+p, i, c]
        xb = xc.transpose(0, 2, 1).reshape(B, CI)
        xb = np.ascontiguousarray(
            xb.reshape(2, 128, CI).transpose(1, 0, 2).reshape(128, 2 * CI)
        )
        # w1[p, k*UO + uo] = W[c, u, o, i]
        w1 = Wc.transpose(0, 3, 1, 2).reshape(CI, UO)
        w1 = np.ascontiguousarray(
            w1.reshape(K, 128, UO).transpose(1, 0, 2).reshape(128, K * UO)
        )
        in_maps.append({"xp": xp, "xb": xb, "w1": w1, "sel": sel, "exp16": exp16})
    return in_maps


def kernel(x, W, _trace=False, _trace_kwargs=None):
    nc = _get_program()
    in_maps = _make_in_maps(x, W)
    res = run_bass_kernel_spmd(
        nc, in_maps, core_ids=list(range(N_CORES)), trace=_trace,
        **(_trace_kwargs or {}),
    )
    out = res.results[0]["out"].astype(np.float32).reshape(B, U, O, 1)
    if _trace:
        kernel.last_results = res
    return out


# revision 12
# speedup vs baseline: 1.1574x; 1.1574x over previous
"""DigitCaps dynamic-routing kernel for 8 TRN2 NeuronCores.

Strategy: shard the C=1152 input capsules across the 8 cores (144 each) and
keep the full batch B=256 on every core.  The routing iterations use the
factored form (never materializing u_hat = x @ W, which would be 189 MB):

  s[b,u,o]    = sum_{c,i} x[b,i,c] * (coef[c,u] * W[c,u,o,i])     (matmul, K=(c,i))
  v           = squash(s)
  G[ci,uo]    = sum_b x[b,i,c] * v[b,u,o]                          (matmul, K=b)
  agr[c,u]    = (1/B) * sum_{o,i} W[c,u,o,i] * G[(c,i),(u,o)]      (mult + selector matmul)
  b_logits   += agr ; coef = softmax_u(b_logits)                   (tiny, c-local)

Only cross-core traffic: AllGather of the per-core partial s ([256,160] f32)
once per routing iteration (4 total).  The c-sharded agreement/logits state is
fully core-local.  Iteration 1's uniform coef=0.1 is folded into a 0.1
pre-scale of the x operand used by the s-matmul (and cancelled for later
iterations by scaling the coefficient-expansion constant by 10).

Precision: matmuls that only feed the routing coefficients (s for iters 0-2,
G, with free dims padded to 256) run as float32r (1 cyc/row at N>=256 vs 4
for fp32).  The final iteration's s-matmul and the coefficient expansion stay
full fp32 since they hit the output directly.
"""

import sys

if "/opt/trn_rl_repo" not in sys.path:
    sys.path.insert(0, "/opt/trn_rl_repo")

import numpy as np

import concourse.bacc as bacc
import concourse.tile as tile
from concourse import mybir
from concourse.bass_utils import run_bass_kernel_spmd

F32 = mybir.dt.float32
F32R = mybir.dt.float32r

B = 256          # batch
IU = 8           # in_unit (i)
C = 1152         # input capsules
U = 10           # output capsules
O = 16           # unit size
N_CORES = 8
CL = C // N_CORES          # 144 local capsules
CI = CL * IU               # 1152 local (c,i) rows
K = CI // 128              # 9 contraction tiles
UO = U * O                 # 160
UOP = 256                  # padded free dim so float32r runs at 1 cyc/row
NROUTE = 4

# matmul dtype for the coefficient-path matmuls (s iters 0-2, G)
FAST_LAST_S = False  # keep the output-facing s-matmul in full fp32


def _mm(nc, out, lhsT, rhs, start, stop, fast):
    if fast:
        lhsT = lhsT.bitcast(F32R)
        rhs = rhs.bitcast(F32R)
    nc.tensor.matmul(out, lhsT=lhsT, rhs=rhs, start=start, stop=stop)


def _build_program():
    nc = bacc.Bacc(
        "TRN2",
        target_bir_lowering=False,
        debug=False,
        enable_asserts=False,
        num_devices=N_CORES,
    )

    xp_d = nc.dram_tensor("xp", [128, K * B], F32, kind="ExternalInput").ap()
    xb_d = nc.dram_tensor("xb", [128, 2 * CI], F32, kind="ExternalInput").ap()
    w1_d = nc.dram_tensor("w1", [128, K * UOP], F32, kind="ExternalInput").ap()
    zc_d = nc.dram_tensor("zc", [128, UOP - UO], F32, kind="ExternalInput").ap()
    sel_d = nc.dram_tensor("sel", [128, 16], F32, kind="ExternalInput").ap()
    exp_d = nc.dram_tensor("exp16", [16, 128], F32, kind="ExternalInput").ap()
    out_d = nc.dram_tensor("out", [B, U, O, 1], F32, kind="ExternalOutput").ap()

    with tile.TileContext(nc) as tc:
        with (
            tc.tile_pool(name="persist", bufs=1) as pp,
            tc.tile_pool(name="work", bufs=2) as wp,
            tc.tile_pool(name="sps", bufs=2, space="PSUM") as sps,
            tc.tile_pool(name="gps", bufs=3, space="PSUM") as gps,
            tc.tile_pool(name="aps", bufs=1, space="PSUM") as aps,
            tc.tile_pool(name="cxps", bufs=1, space="PSUM") as cxps,
            tc.tile_pool(name="dram", bufs=2, space="DRAM") as dram,
        ):
            # ---- persistent tiles; padded regions zeroed once ----
            xp_s = pp.tile([128, K * B], F32, tag="xp")
            xb_s = pp.tile([128, 2 * CI], F32, tag="xb")
            w1_s = pp.tile([128, K * UOP], F32, tag="w1")     # [.., k*256+uo], pad 160:256
            weff_s = pp.tile([128, K * UOP], F32, tag="weff")
            v_s = pp.tile([128, 2 * UOP], F32, tag="v")       # [.., t*256+uo]
            v_last = pp.tile([128, 2 * UO], F32, tag="vlast")
            weff_last = pp.tile([128, K * UO], F32, tag="wefflast")
            pm2_s = pp.tile([128, K * U], F32, tag="pm2")     # o-reduced G*W
            sel_s = pp.tile([128, 16], F32, tag="sel")
            exp_s = pp.tile([16, 128], F32, tag="exp16")
            b_state = pp.tile([16, K * U], F32, tag="bstate")
            scr = pp.tile([128, 2], F32, tag="scr")           # ACT table prewarm scratch

            nc.gpsimd.memset(scr[:], 1.0)

            for k in range(K):
                nc.sync.dma_start(
                    xp_s[:, k * B:(k + 1) * B].bitcast(F32R),
                    xp_d[:, k * B:(k + 1) * B].bitcast(F32R),
                )
                nc.sync.dma_start(
                    w1_s[:, k * UOP:(k + 1) * UOP].bitcast(F32R),
                    w1_d[:, k * UOP:(k + 1) * UOP].bitcast(F32R),
                )
                nc.sync.dma_start(
                    weff_s[:, k * UOP + UO:(k + 1) * UOP].bitcast(F32R),
                    zc_d.bitcast(F32R),
                )
            for t in range(2):
                nc.sync.dma_start(
                    v_s[:, t * UOP + UO:(t + 1) * UOP].bitcast(F32R),
                    zc_d.bitcast(F32R),
                )
            nc.sync.dma_start(xb_s[:, :CI].bitcast(F32R), xb_d[:, :CI].bitcast(F32R))
            nc.sync.dma_start(xb_s[:, CI:].bitcast(F32R), xb_d[:, CI:].bitcast(F32R))
            nc.sync.dma_start(sel_s[:], sel_d)
            nc.sync.dma_start(exp_s[:], exp_d)

            weff = w1_s  # iteration 0 uses raw W (coef folded into xp scale)

            for r in range(NROUTE):
                last = r == NROUTE - 1
                fast_s = not last or FAST_LAST_S
                # ---- s partial: [b,(u,o)] += xp^T @ weff over (c,i) tiles ----
                s_stage = wp.tile([128, 2 * UO], F32, tag="s_stage")
                wstride = UOP if fast_s else UO
                for g in range(2):
                    s_ps = sps.tile([128, UOP], F32, tag="s_ps")
                    for k in range(K):
                        _mm(
                            nc, s_ps[:] if fast_s else s_ps[:, :UO],
                            xp_s[:, k * B + g * 128: k * B + (g + 1) * 128],
                            weff[:, k * wstride: k * wstride + wstride],
                            start=(k == 0), stop=(k == K - 1), fast=fast_s,
                        )
                    nc.vector.tensor_copy(s_stage[:, g * UO:(g + 1) * UO], s_ps[:, :UO])

                # ---- AllGather partials, local 8-way tree sum ----
                cc_in = dram.tile([B, UO], F32, tag="cc_in")
                cc_out = dram.tile(
                    [N_CORES * B, UO], F32, tag="cc_out", addr_space="Shared"
                )
                nc.sync.dma_start(
                    cc_in.opt().rearrange("(g p) f -> p g f", g=2), s_stage[:]
                )
                nc.gpsimd.collective_compute(
                    "AllGather",
                    mybir.AluOpType.bypass,
                    replica_groups=[list(range(N_CORES))],
                    ins=[cc_in.opt()],
                    outs=[cc_out.opt()],
                )
                sg_s = wp.tile([128, N_CORES * 2 * UO], F32, tag="sg")
                nc.sync.dma_start(
                    sg_s[:], cc_out.opt().rearrange("(r g p) f -> p (r g) f", r=N_CORES, g=2)
                )
                t1 = wp.tile([128, 4 * 2 * UO], F32, tag="t1")
                nc.vector.tensor_add(t1[:], sg_s[:, :4 * 2 * UO], sg_s[:, 4 * 2 * UO:])
                t2 = wp.tile([128, 2 * 2 * UO], F32, tag="t2")
                nc.vector.tensor_add(t2[:], t1[:, :2 * 2 * UO], t1[:, 2 * 2 * UO:])
                s_sb = wp.tile([128, 2 * UO], F32, tag="s_sb")
                nc.vector.tensor_add(s_sb[:], t2[:, :2 * UO], t2[:, 2 * UO:])

                # ---- squash: v = s * sqrt(n2) / (1 + n2) ----
                sq = wp.tile([128, 2 * UO], F32, tag="sq")
                nc.vector.tensor_mul(sq[:], s_sb[:], s_sb[:])
                n2 = wp.tile([128, 2 * U], F32, tag="n2")
                nc.vector.reduce_sum(
                    n2[:], sq[:].rearrange("p (t u o) -> p (t u) o", t=2, u=U),
                    axis=mybir.AxisListType.X,
                )
                rt = wp.tile([128, 2 * U], F32, tag="rt")
                nc.scalar.sqrt(rt[:], n2[:])
                if not last:
                    # prewarm the Exp ACT table while G/sel run (dep on rt orders it)
                    nc.scalar.activation(
                        scr[:, 1:2], rt[:, 0:1], mybir.ActivationFunctionType.Exp
                    )
                dn = wp.tile([128, 2 * U], F32, tag="dn")
                nc.vector.tensor_scalar_add(dn[:], n2[:], 1.0)
                rd = wp.tile([128, 2 * U], F32, tag="rd")
                nc.vector.reciprocal(rd[:], dn[:])
                f = wp.tile([128, 2 * U], F32, tag="f")
                nc.vector.tensor_mul(f[:], rt[:], rd[:])
                if last:
                    v_out = v_last[:].rearrange("p (t u o) -> p t u o", t=2, u=U)
                else:
                    v_out = v_s[:].rearrange("p (t q o) -> p t q o", t=2, q=16)[:, :, :U, :].bitcast(F32R)
                nc.vector.tensor_mul(
                    v_out,
                    s_sb[:].rearrange("p (t u o) -> p t u o", t=2, u=U),
                    f[:].rearrange("p (t u) -> p t u", t=2).unsqueeze(3).broadcast_to((128, 2, U, O)),
                )

                if last:
                    nc.sync.dma_start(
                        out_d.rearrange("(g p) u o one -> p g (u o one)", g=2),
                        v_last[:].rearrange("p (t f) -> p t f", t=2),
                    )
                    break

                # ---- G[(c,i),(u,o)] = sum_b x*v ; pm2 = sum_o G*W ; agr = sel^T pm2 ----
                for m in range(K):
                    g_ps = gps.tile([128, UOP], F32, tag="g_ps")
                    for t in range(2):
                        _mm(
                            nc, g_ps[:],
                            xb_s[:, t * CI + m * 128: t * CI + (m + 1) * 128],
                            v_s[:, t * UOP:(t + 1) * UOP],
                            start=(t == 0), stop=(t == 1), fast=True,
                        )
                    pm = wp.tile([128, UO], F32, tag="pm")
                    nc.vector.tensor_mul(pm[:], g_ps[:, :UO], w1_s[:, m * UOP: m * UOP + UO])
                    nc.vector.reduce_sum(
                        pm2_s[:, m * U:(m + 1) * U],
                        pm[:].rearrange("p (u o) -> p u o", u=U),
                        axis=mybir.AxisListType.X,
                    )
                a_ps = aps.tile([16, K * U], F32, tag="a_ps")
                nc.tensor.matmul(a_ps[:], lhsT=sel_s[:], rhs=pm2_s[:], start=True, stop=True)

                # ---- logits update + softmax over u (c-local, tiny) ----
                if r == 0:
                    nc.vector.tensor_copy(b_state[:], a_ps[:])
                else:
                    nc.vector.tensor_add(b_state[:], b_state[:], a_ps[:])
                eb = wp.tile([16, K * U], F32, tag="eb")
                nc.scalar.activation(eb[:], b_state[:], mybir.ActivationFunctionType.Exp)
                # prewarm the Sqrt ACT table for the next squash
                nc.scalar.activation(
                    scr[:16, 0:1], eb[:, 0:1], mybir.ActivationFunctionType.Sqrt
                )
                den = wp.tile([16, K], F32, tag="den")
                nc.vector.reduce_sum(
                    den[:], eb[:].rearrange("p (k u) -> p k u", k=K),
                    axis=mybir.AxisListType.X,
                )
                rden = wp.tile([16, K], F32, tag="rden")
                nc.vector.reciprocal(rden[:], den[:])
                cnorm = wp.tile([16, K * U], F32, tag="cnorm")
                nc.vector.tensor_mul(
                    cnorm[:].rearrange("p (k u) -> p k u", k=K),
                    eb[:].rearrange("p (k u) -> p k u", k=K),
                    rden[:].unsqueeze(2).broadcast_to((16, K, U)),
                )

                # ---- expand coef to (c,i) partitions; W_eff = W * coef ----
                cx_ps = cxps.tile([128, K * U], F32, tag="cx")
                nc.tensor.matmul(cx_ps[:], lhsT=exp_s[:], rhs=cnorm[:], start=True, stop=True)
                cx_sb = wp.tile([128, K * U], F32, tag="cx_sb")
                nc.vector.tensor_copy(cx_sb[:], cx_ps[:])
                if r < NROUTE - 2 or FAST_LAST_S:
                    weff_out = weff_s[:].rearrange("p (k q o) -> p k q o", k=K, q=16)[:, :, :U, :].bitcast(F32R)
                    weff = weff_s
                else:
                    weff_out = weff_last[:].rearrange("p (k u o) -> p k u o", k=K, u=U)
                    weff = weff_last
                nc.vector.tensor_mul(
                    weff_out,
                    w1_s[:].rearrange("p (k q o) -> p k q o", k=K, q=16)[:, :, :U, :],
                    cx_sb[:].rearrange("p (k u) -> p k u", k=K).unsqueeze(3).broadcast_to((128, K, U, O)),
                )

    nc.compile()
    return nc


_PROGRAM_CACHE = {}


def _get_program():
    if "nc" not in _PROGRAM_CACHE:
        _PROGRAM_CACHE["nc"] = _build_program()
    return _PROGRAM_CACHE["nc"]


def _make_in_maps(x, W):
    x = np.ascontiguousarray(x, dtype=np.float32)
    W = np.ascontiguousarray(W, dtype=np.float32)
    sel = np.zeros((128, 16), dtype=np.float32)
    for p in range(128):
        sel[p, p // IU] = 1.0 / B
    exp16 = np.zeros((16, 128), dtype=np.float32)
    for p in range(128):
        exp16[p // IU, p] = 10.0  # cancels the 0.1 pre-scale of xp

    in_maps = []
    for core in range(N_CORES):
        c0 = core * CL
        xc = x[:, :, c0:c0 + CL]                    # [B, I, CL]
        Wc = W[c0:c0 + CL]                          # [CL, U, O, I]
        # xp[p, k*B + b] = 0.1 * x[b, i, c], ci = k*128+p = c_rel*8+i
        xp = 0.1 * xc.transpose(2, 1, 0).reshape(CI, B)
        xp = np.ascontiguousarray(
            xp.reshape(K, 128, B).transpose(1, 0, 2).reshape(128, K * B)
        )
        # xb[p, t*CI + ci] = x[t*128+p, i, c]
        xb = xc.transpose(0, 2, 1).reshape(B, CI)
        xb = np.ascontiguousarray(
            xb.reshape(2, 128, CI).transpose(1, 0, 2).reshape(128, 2 * CI)
        )
        # w1[p, k*UOP + uo] = W[c, u, o, i], zero-padded to UOP per k-tile
        w1 = Wc.transpose(0, 3, 1, 2).reshape(CI, UO).reshape(K, 128, UO)
        w1p = np.zeros((128, K, UOP), dtype=np.float32)
        w1p[:, :, :UO] = w1.transpose(1, 0, 2)
        w1p = np.ascontiguousarray(w1p.reshape(128, K * UOP))
        zc = np.zeros((128, UOP - UO), dtype=np.float32)
        in_maps.append(
            {"xp": xp, "xb": xb, "w1": w1p, "sel": sel, "exp16": exp16, "zc": zc}
        )
    return in_maps


def kernel(x, W, _trace=False, _trace_kwargs=None):
    nc = _get_program()
    in_maps = _make_in_maps(x, W)
    res = run_bass_kernel_spmd(
        nc, in_maps, core_ids=list(range(N_CORES)), trace=_trace,
        **(_trace_kwargs or {}),
    )
    out = res.results[0]["out"].astype(np.float32).reshape(B, U, O, 1)
    if _trace:
        kernel.last_results = res
    return out


# revision 19
# speedup vs baseline: 1.1793x; 1.0189x over previous
"""DigitCaps dynamic-routing kernel for 8 TRN2 NeuronCores.

Strategy: shard the C=1152 input capsules across the 8 cores (144 each) and
keep the full batch B=256 on every core.  The routing iterations use the
factored form (never materializing u_hat = x @ W, which would be 189 MB):

  s[b,u,o]    = sum_{c,i} x[b,i,c] * (coef[c,u] * W[c,u,o,i])     (matmul, K=(c,i))
  v           = squash(s)
  G[ci,uo]    = sum_b x[b,i,c] * v[b,u,o]                          (matmul, K=b)
  agr[c,u]    = (1/B) * sum_{o,i} W[c,u,o,i] * G[(c,i),(u,o)]      (mult + selector matmul)
  b_logits   += agr ; coef = softmax_u(b_logits)                   (tiny, c-local)

Only cross-core traffic: AllGather of the per-core partial s ([256,160] f32)
once per routing iteration (4 total).  The c-sharded agreement/logits state is
fully core-local.  Iteration 1's uniform coef=0.1 is folded into a 0.1
pre-scale of the x operand used by the s-matmul (and cancelled for later
iterations by scaling the coefficient-expansion constant by 10).

Precision: matmuls that only feed the routing coefficients (s for iters 0-2,
G, with free dims padded to 256) run as float32r (1 cyc/row at N>=256 vs 4
for fp32).  The final iteration's s-matmul and the coefficient expansion stay
full fp32 since they hit the output directly.
"""

import sys

if "/opt/trn_rl_repo" not in sys.path:
    sys.path.insert(0, "/opt/trn_rl_repo")

import numpy as np

import concourse.bacc as bacc
import concourse.tile as tile
from concourse import mybir
from concourse.bass_utils import run_bass_kernel_spmd

F32 = mybir.dt.float32
F32R = mybir.dt.float32r

B = 256          # batch
IU = 8           # in_unit (i)
C = 1152         # input capsules
U = 10           # output capsules
O = 16           # unit size
N_CORES = 8
CL = C // N_CORES          # 144 local capsules
CI = CL * IU               # 1152 local (c,i) rows
K = CI // 128              # 9 contraction tiles
UO = U * O                 # 160
UOP = 256                  # padded free dim so float32r runs at 1 cyc/row
NROUTE = 4
N_WARM = 24   # PE keep-warm dummy matmuls per routing iteration

# matmul dtype for the coefficient-path matmuls (s iters 0-2, G)
FAST_LAST_S = False  # keep the output-facing s-matmul in full fp32


def _mm(nc, out, lhsT, rhs, start, stop, fast):
    if fast:
        lhsT = lhsT.bitcast(F32R)
        rhs = rhs.bitcast(F32R)
    nc.tensor.matmul(out, lhsT=lhsT, rhs=rhs, start=start, stop=stop)


def _build_program():
    nc = bacc.Bacc(
        "TRN2",
        target_bir_lowering=False,
        debug=False,
        enable_asserts=False,
        num_devices=N_CORES,
    )

    xp_d = nc.dram_tensor("xp", [128, K * B], F32, kind="ExternalInput").ap()
    xb_d = nc.dram_tensor("xb", [128, 2 * CI], F32, kind="ExternalInput").ap()
    w1_d = nc.dram_tensor("w1", [128, K * UOP], F32, kind="ExternalInput").ap()
    zc_d = nc.dram_tensor("zc", [128, K * (UOP - UO)], F32, kind="ExternalInput").ap()
    sel_d = nc.dram_tensor("sel", [128, 16], F32, kind="ExternalInput").ap()
    exp_d = nc.dram_tensor("exp16", [16, 128], F32, kind="ExternalInput").ap()
    out_d = nc.dram_tensor("out", [B, U, O, 1], F32, kind="ExternalOutput").ap()

    with tile.TileContext(nc) as tc:
        with (
            tc.tile_pool(name="persist", bufs=1) as pp,
            tc.tile_pool(name="work", bufs=2) as wp,
            tc.tile_pool(name="sps", bufs=2, space="PSUM") as sps,
            tc.tile_pool(name="gps", bufs=3, space="PSUM") as gps,
            tc.tile_pool(name="aps", bufs=1, space="PSUM") as aps,
            tc.tile_pool(name="cxps", bufs=1, space="PSUM") as cxps,
            tc.tile_pool(name="dram", bufs=2, space="DRAM") as dram,
        ):
            # ---- persistent tiles; padded regions zeroed once ----
            xp_s = pp.tile([128, K * B], F32, tag="xp")
            xb_s = pp.tile([128, 2 * CI], F32, tag="xb")
            w1_s = pp.tile([128, K * UOP], F32, tag="w1")     # [.., k*256+uo], pad 160:256
            weff_s = pp.tile([128, K * UOP], F32, tag="weff")
            v_s = pp.tile([128, 2 * UOP], F32, tag="v")       # [.., t*256+uo]
            v_last = pp.tile([128, 2 * UO], F32, tag="vlast")
            weff_last = pp.tile([128, K * UO], F32, tag="wefflast")
            pm2_s = pp.tile([128, K * U], F32, tag="pm2")     # o-reduced G*W
            sel_s = pp.tile([128, 16], F32, tag="sel")
            exp_s = pp.tile([16, 128], F32, tag="exp16")
            b_state = pp.tile([16, K * U], F32, tag="bstate")
            scr = pp.tile([128, 2], F32, tag="scr")           # ACT table prewarm scratch

            nc.gpsimd.memset(scr[:], 1.0)


            for j in range(3):
                kb3, ku3 = 3 * B, 3 * UOP
                nc.sync.dma_start(
                    xp_s[:, j * kb3:(j + 1) * kb3].bitcast(F32R),
                    xp_d[:, j * kb3:(j + 1) * kb3].bitcast(F32R),
                )
                nc.scalar.dma_start(
                    w1_s[:, j * ku3:(j + 1) * ku3].bitcast(F32R),
                    w1_d[:, j * ku3:(j + 1) * ku3].bitcast(F32R),
                )
            nc.scalar.dma_start(
                weff_s[:].rearrange("p (k q) -> p k q", k=K)[:, :, UO:].bitcast(F32R),
                zc_d.rearrange("p (k q) -> p k q", k=K).bitcast(F32R),
            )
            nc.scalar.dma_start(
                v_s[:].rearrange("p (t q) -> p t q", t=2)[:, :, UO:].bitcast(F32R),
                zc_d[:, : 2 * (UOP - UO)].rearrange("p (t q) -> p t q", t=2).bitcast(F32R),
            )
            nc.sync.dma_start(xb_s[:, :CI].bitcast(F32R), xb_d[:, :CI].bitcast(F32R))
            nc.sync.dma_start(xb_s[:, CI:].bitcast(F32R), xb_d[:, CI:].bitcast(F32R))
            nc.scalar.dma_start(sel_s[:], sel_d)
            nc.scalar.dma_start(exp_s[:], exp_d)

            weff = w1_s  # iteration 0 uses raw W (coef folded into xp scale)

            for r in range(NROUTE):
                last = r == NROUTE - 1
                fast_s = not last or FAST_LAST_S
                # ---- s partial: [b,(u,o)] += xp^T @ weff over (c,i) tiles ----
                s_stage = wp.tile([128, 2 * UO], F32, tag="s_stage")
                wstride = UOP if fast_s else UO
                for g in range(2):
                    s_ps = sps.tile([128, UOP], F32, tag="s_ps")
                    for k in range(K):
                        _mm(
                            nc, s_ps[:] if fast_s else s_ps[:, :UO],
                            xp_s[:, k * B + g * 128: k * B + (g + 1) * 128],
                            weff[:, k * wstride: k * wstride + wstride],
                            start=(k == 0), stop=(k == K - 1), fast=fast_s,
                        )
                    nc.vector.tensor_copy(s_stage[:, g * UO:(g + 1) * UO], s_ps[:, :UO])

                # ---- AllGather partials, local 8-way tree sum ----
                cc_in = dram.tile([B, UO], F32, tag="cc_in")
                cc_out = dram.tile(
                    [N_CORES * B, UO], F32, tag="cc_out", addr_space="Shared"
                )
                nc.sync.dma_start(
                    cc_in.opt().rearrange("(g p) f -> p g f", g=2), s_stage[:]
                )
                nc.gpsimd.collective_compute(
                    "AllGather",
                    mybir.AluOpType.bypass,
                    replica_groups=[list(range(N_CORES))],
                    ins=[cc_in.opt()],
                    outs=[cc_out.opt()],
                )
                sg_s = wp.tile([128, N_CORES * 2 * UO], F32, tag="sg")
                nc.sync.dma_start(
                    sg_s[:], cc_out.opt().rearrange("(r g p) f -> p (r g) f", r=N_CORES, g=2)
                )
                t1 = wp.tile([128, 4 * 2 * UO], F32, tag="t1")
                nc.vector.tensor_add(t1[:], sg_s[:, :4 * 2 * UO], sg_s[:, 4 * 2 * UO:])
                t2 = wp.tile([128, 2 * 2 * UO], F32, tag="t2")
                nc.vector.tensor_add(t2[:], t1[:, :2 * 2 * UO], t1[:, 2 * 2 * UO:])
                s_sb = wp.tile([128, 2 * UO], F32, tag="s_sb")
                nc.vector.tensor_add(s_sb[:], t2[:, :2 * UO], t2[:, 2 * UO:])

                # ---- squash: v = s * sqrt(n2) / (1 + n2) ----
                sq = wp.tile([128, 2 * UO], F32, tag="sq")
                nc.vector.tensor_mul(sq[:], s_sb[:], s_sb[:])
                n2 = wp.tile([128, 2 * U], F32, tag="n2")
                nc.vector.reduce_sum(
                    n2[:], sq[:].rearrange("p (t u o) -> p (t u) o", t=2, u=U),
                    axis=mybir.AxisListType.X,
                )
                rt = wp.tile([128, 2 * U], F32, tag="rt")
                nc.scalar.sqrt(rt[:], n2[:])
                if not last:
                    # prewarm the Exp ACT table while G/sel run (dep on rt orders it)
                    nc.scalar.activation(
                        scr[:, 1:2], rt[:, 0:1], mybir.ActivationFunctionType.Exp
                    )
                dn = wp.tile([128, 2 * U], F32, tag="dn")
                nc.vector.tensor_scalar_add(dn[:], n2[:], 1.0)
                rd = wp.tile([128, 2 * U], F32, tag="rd")
                nc.vector.reciprocal(rd[:], dn[:])
                f = wp.tile([128, 2 * U], F32, tag="f")
                nc.vector.tensor_mul(f[:], rt[:], rd[:])
                if last:
                    v_out = v_last[:].rearrange("p (t u o) -> p t u o", t=2, u=U)
                else:
                    v_out = v_s[:].rearrange("p (t q o) -> p t q o", t=2, q=16)[:, :, :U, :].bitcast(F32R)
                nc.vector.tensor_mul(
                    v_out,
                    s_sb[:].rearrange("p (t u o) -> p t u o", t=2, u=U),
                    f[:].rearrange("p (t u) -> p t u", t=2).unsqueeze(3).broadcast_to((128, 2, U, O)),
                )

                if last:
                    nc.sync.dma_start(
                        out_d.rearrange("(g p) u o one -> p g (u o one)", g=2),
                        v_last[:].rearrange("p (t f) -> p t f", t=2),
                    )
                    break

                # ---- G[(c,i),(u,o)] = sum_b x*v ; pm2 = sum_o G*W ; agr = sel^T pm2 ----
                for m in range(K):
                    g_ps = gps.tile([128, UOP], F32, tag="g_ps")
                    for t in range(2):
                        _mm(
                            nc, g_ps[:],
                            xb_s[:, t * CI + m * 128: t * CI + (m + 1) * 128],
                            v_s[:, t * UOP:(t + 1) * UOP],
                            start=(t == 0), stop=(t == 1), fast=True,
                        )
                    pm = wp.tile([128, UO], F32, tag="pm")
                    nc.vector.tensor_mul(pm[:], g_ps[:, :UO], w1_s[:, m * UOP: m * UOP + UO])
                    nc.vector.reduce_sum(
                        pm2_s[:, m * U:(m + 1) * U],
                        pm[:].rearrange("p (u o) -> p u o", u=U),
                        axis=mybir.AxisListType.X,
                    )
                a_ps = aps.tile([16, K * U], F32, tag="a_ps")
                nc.tensor.matmul(a_ps[:], lhsT=sel_s[:], rhs=pm2_s[:], start=True, stop=True)

                # ---- logits update + softmax over u (c-local, tiny) ----
                if r == 0:
                    nc.vector.tensor_copy(b_state[:], a_ps[:])
                else:
                    nc.vector.tensor_add(b_state[:], b_state[:], a_ps[:])
                eb = wp.tile([16, K * U], F32, tag="eb")
                nc.scalar.activation(eb[:], b_state[:], mybir.ActivationFunctionType.Exp)
                # prewarm the Sqrt ACT table for the next squash
                nc.scalar.activation(
                    scr[:16, 0:1], eb[:, 0:1], mybir.ActivationFunctionType.Sqrt
                )
                den = wp.tile([16, K], F32, tag="den")
                nc.vector.reduce_sum(
                    den[:], eb[:].rearrange("p (k u) -> p k u", k=K),
                    axis=mybir.AxisListType.X,
                )
                rden = wp.tile([16, K], F32, tag="rden")
                nc.vector.reciprocal(rden[:], den[:])
                cnorm = wp.tile([16, K * U], F32, tag="cnorm")
                nc.vector.tensor_mul(
                    cnorm[:].rearrange("p (k u) -> p k u", k=K),
                    eb[:].rearrange("p (k u) -> p k u", k=K),
                    rden[:].unsqueeze(2).broadcast_to((16, K, U)),
                )

                # ---- expand coef to (c,i) partitions; W_eff = W * coef ----
                cx_ps = cxps.tile([128, K * U], F32, tag="cx")
                nc.tensor.matmul(cx_ps[:], lhsT=exp_s[:], rhs=cnorm[:], start=True, stop=True)
                cx_sb = wp.tile([128, K * U], F32, tag="cx_sb")
                nc.vector.tensor_copy(cx_sb[:], cx_ps[:])
                if r < NROUTE - 2 or FAST_LAST_S:
                    weff_out = weff_s[:].rearrange("p (k q o) -> p k q o", k=K, q=16)[:, :, :U, :].bitcast(F32R)
                    weff = weff_s
                else:
                    weff_out = weff_last[:].rearrange("p (k u o) -> p k u o", k=K, u=U)
                    weff = weff_last
                nc.vector.tensor_mul(
                    weff_out,
                    w1_s[:].rearrange("p (k q o) -> p k q o", k=K, q=16)[:, :, :U, :],
                    cx_ps[:].rearrange("p (k u) -> p k u", k=K).unsqueeze(3).broadcast_to((128, K, U, O)),
                )

    nc.compile()
    return nc


_PROGRAM_CACHE = {}


def _get_program():
    if "nc" not in _PROGRAM_CACHE:
        _PROGRAM_CACHE["nc"] = _build_program()
    return _PROGRAM_CACHE["nc"]


def _make_in_maps(x, W):
    x = np.ascontiguousarray(x, dtype=np.float32)
    W = np.ascontiguousarray(W, dtype=np.float32)
    sel = np.zeros((128, 16), dtype=np.float32)
    for p in range(128):
        sel[p, p // IU] = 1.0 / B
    exp16 = np.zeros((16, 128), dtype=np.float32)
    for p in range(128):
        exp16[p // IU, p] = 10.0  # cancels the 0.1 pre-scale of xp

    in_maps = []
    for core in range(N_CORES):
        c0 = core * CL
        xc = x[:, :, c0:c0 + CL]                    # [B, I, CL]
        Wc = W[c0:c0 + CL]                          # [CL, U, O, I]
        # xp[p, k*B + b] = 0.1 * x[b, i, c], ci = k*128+p = c_rel*8+i
        xp = 0.1 * xc.transpose(2, 1, 0).reshape(CI, B)
        xp = np.ascontiguousarray(
            xp.reshape(K, 128, B).transpose(1, 0, 2).reshape(128, K * B)
        )
        # xb[p, t*CI + ci] = x[t*128+p, i, c]
        xb = xc.transpose(0, 2, 1).reshape(B, CI)
        xb = np.ascontiguousarray(
            xb.reshape(2, 128, CI).transpose(1, 0, 2).reshape(128, 2 * CI)
        )
        # w1[p, k*UOP + uo] = W[c, u, o, i], zero-padded to UOP per k-tile
        w1 = Wc.transpose(0, 3, 1, 2).reshape(CI, UO).reshape(K, 128, UO)
        w1p = np.zeros((128, K, UOP), dtype=np.float32)
        w1p[:, :, :UO] = w1.transpose(1, 0, 2)
        w1p = np.ascontiguousarray(w1p.reshape(128, K * UOP))
        zc = np.zeros((128, K * (UOP - UO)), dtype=np.float32)
        in_maps.append(
            {"xp": xp, "xb": xb, "w1": w1p, "sel": sel, "exp16": exp16, "zc": zc}
        )
    return in_maps


def kernel(x, W, _trace=False, _trace_kwargs=None):
    nc = _get_program()
    in_maps = _make_in_maps(x, W)
    res = run_bass_kernel_spmd(
        nc, in_maps, core_ids=list(range(N_CORES)), trace=_trace,
        **(_trace_kwargs or {}),
    )
    out = res.results[0]["out"].astype(np.float32).reshape(B, U, O, 1)
    if _trace:
        kernel.last_results = res
    return out


# revision 20
# speedup vs baseline: 1.2629x; 1.0709x over previous
"""DigitCaps dynamic-routing kernel for 8 TRN2 NeuronCores.

Strategy: shard the C=1152 input capsules across the 8 cores (144 each) and
keep the full batch B=256 on every core.  The routing iterations use the
factored form (never materializing u_hat = x @ W, which would be 189 MB):

  s[b,u,o]    = sum_{c,i} x[b,i,c] * (coef[c,u] * W[c,u,o,i])     (matmul, K=(c,i))
  v           = squash(s)
  G[ci,uo]    = sum_b x[b,i,c] * v[b,u,o]                          (matmul, K=b)
  agr[c,u]    = (1/B) * sum_{o,i} W[c,u,o,i] * G[(c,i),(u,o)]      (mult + selector matmul)
  b_logits   += agr ; coef = softmax_u(b_logits)                   (tiny, c-local)

Only cross-core traffic: AllGather of the per-core partial s ([256,160] f32)
once per routing iteration (4 total).  The c-sharded agreement/logits state is
fully core-local.  Iteration 1's uniform coef=0.1 is folded into a 0.1
pre-scale of the x operand used by the s-matmul (and cancelled for later
iterations by scaling the coefficient-expansion constant by 10).

Precision: matmuls that only feed the routing coefficients (s for iters 0-2,
G, with free dims padded to 256) run as float32r (1 cyc/row at N>=256 vs 4
for fp32).  The final iteration's s-matmul and the coefficient expansion stay
full fp32 since they hit the output directly.
"""

import sys

if "/opt/trn_rl_repo" not in sys.path:
    sys.path.insert(0, "/opt/trn_rl_repo")

import numpy as np

import concourse.bacc as bacc
import concourse.tile as tile
from concourse import mybir
from concourse.bass_utils import run_bass_kernel_spmd

F32 = mybir.dt.float32
F32R = mybir.dt.float32r

B = 256          # batch
IU = 8           # in_unit (i)
C = 1152         # input capsules
U = 10           # output capsules
O = 16           # unit size
N_CORES = 8
CL = C // N_CORES          # 144 local capsules
CI = CL * IU               # 1152 local (c,i) rows
K = CI // 128              # 9 contraction tiles
UO = U * O                 # 160
UOP = 256                  # padded free dim so float32r runs at 1 cyc/row
NROUTE = 4
N_WARM = 24   # PE keep-warm dummy matmuls per routing iteration

# matmul dtype for the coefficient-path matmuls (s iters 0-2, G)
FAST_LAST_S = True  # fp32r on the final s-matmul too (~4us tail saving, ~+2e-4 err)


def _mm(nc, out, lhsT, rhs, start, stop, fast):
    if fast:
        lhsT = lhsT.bitcast(F32R)
        rhs = rhs.bitcast(F32R)
    nc.tensor.matmul(out, lhsT=lhsT, rhs=rhs, start=start, stop=stop)


def _build_program():
    nc = bacc.Bacc(
        "TRN2",
        target_bir_lowering=False,
        debug=False,
        enable_asserts=False,
        num_devices=N_CORES,
    )

    xp_d = nc.dram_tensor("xp", [128, K * B], F32, kind="ExternalInput").ap()
    xb_d = nc.dram_tensor("xb", [128, 2 * CI], F32, kind="ExternalInput").ap()
    w1_d = nc.dram_tensor("w1", [128, K * UOP], F32, kind="ExternalInput").ap()
    zc_d = nc.dram_tensor("zc", [128, K * (UOP - UO)], F32, kind="ExternalInput").ap()
    sel_d = nc.dram_tensor("sel", [128, 16], F32, kind="ExternalInput").ap()
    exp_d = nc.dram_tensor("exp16", [16, 128], F32, kind="ExternalInput").ap()
    out_d = nc.dram_tensor("out", [B, U, O, 1], F32, kind="ExternalOutput").ap()

    with tile.TileContext(nc) as tc:
        with (
            tc.tile_pool(name="persist", bufs=1) as pp,
            tc.tile_pool(name="work", bufs=2) as wp,
            tc.tile_pool(name="sps", bufs=2, space="PSUM") as sps,
            tc.tile_pool(name="gps", bufs=3, space="PSUM") as gps,
            tc.tile_pool(name="aps", bufs=1, space="PSUM") as aps,
            tc.tile_pool(name="cxps", bufs=1, space="PSUM") as cxps,
            tc.tile_pool(name="dram", bufs=2, space="DRAM") as dram,
        ):
            # ---- persistent tiles; padded regions zeroed once ----
            xp_s = pp.tile([128, K * B], F32, tag="xp")
            xb_s = pp.tile([128, 2 * CI], F32, tag="xb")
            w1_s = pp.tile([128, K * UOP], F32, tag="w1")     # [.., k*256+uo], pad 160:256
            weff_s = pp.tile([128, K * UOP], F32, tag="weff")
            v_s = pp.tile([128, 2 * UOP], F32, tag="v")       # [.., t*256+uo]
            v_last = pp.tile([128, 2 * UO], F32, tag="vlast")
            weff_last = pp.tile([128, K * UO], F32, tag="wefflast")
            pm2_s = pp.tile([128, K * U], F32, tag="pm2")     # o-reduced G*W
            sel_s = pp.tile([128, 16], F32, tag="sel")
            exp_s = pp.tile([16, 128], F32, tag="exp16")
            b_state = pp.tile([16, K * U], F32, tag="bstate")
            scr = pp.tile([128, 2], F32, tag="scr")           # ACT table prewarm scratch

            nc.gpsimd.memset(scr[:], 1.0)


            for j in range(3):
                kb3, ku3 = 3 * B, 3 * UOP
                nc.sync.dma_start(
                    xp_s[:, j * kb3:(j + 1) * kb3].bitcast(F32R),
                    xp_d[:, j * kb3:(j + 1) * kb3].bitcast(F32R),
                )
                nc.scalar.dma_start(
                    w1_s[:, j * ku3:(j + 1) * ku3].bitcast(F32R),
                    w1_d[:, j * ku3:(j + 1) * ku3].bitcast(F32R),
                )
            nc.scalar.dma_start(
                weff_s[:].rearrange("p (k q) -> p k q", k=K)[:, :, UO:].bitcast(F32R),
                zc_d.rearrange("p (k q) -> p k q", k=K).bitcast(F32R),
            )
            nc.scalar.dma_start(
                v_s[:].rearrange("p (t q) -> p t q", t=2)[:, :, UO:].bitcast(F32R),
                zc_d[:, : 2 * (UOP - UO)].rearrange("p (t q) -> p t q", t=2).bitcast(F32R),
            )
            nc.sync.dma_start(xb_s[:, :CI].bitcast(F32R), xb_d[:, :CI].bitcast(F32R))
            nc.sync.dma_start(xb_s[:, CI:].bitcast(F32R), xb_d[:, CI:].bitcast(F32R))
            nc.scalar.dma_start(sel_s[:], sel_d)
            nc.scalar.dma_start(exp_s[:], exp_d)

            weff = w1_s  # iteration 0 uses raw W (coef folded into xp scale)

            for r in range(NROUTE):
                last = r == NROUTE - 1
                fast_s = not last or FAST_LAST_S
                # ---- s partial: [b,(u,o)] += xp^T @ weff over (c,i) tiles ----
                s_stage = wp.tile([128, 2 * UO], F32, tag="s_stage")
                wstride = UOP if fast_s else UO
                for g in range(2):
                    s_ps = sps.tile([128, UOP], F32, tag="s_ps")
                    for k in range(K):
                        _mm(
                            nc, s_ps[:] if fast_s else s_ps[:, :UO],
                            xp_s[:, k * B + g * 128: k * B + (g + 1) * 128],
                            weff[:, k * wstride: k * wstride + wstride],
                            start=(k == 0), stop=(k == K - 1), fast=fast_s,
                        )
                    nc.vector.tensor_copy(s_stage[:, g * UO:(g + 1) * UO], s_ps[:, :UO])

                # ---- AllGather partials, local 8-way tree sum ----
                cc_in = dram.tile([B, UO], F32, tag="cc_in")
                cc_out = dram.tile(
                    [N_CORES * B, UO], F32, tag="cc_out", addr_space="Shared"
                )
                nc.sync.dma_start(
                    cc_in.opt().rearrange("(g p) f -> p g f", g=2), s_stage[:]
                )
                nc.gpsimd.collective_compute(
                    "AllGather",
                    mybir.AluOpType.bypass,
                    replica_groups=[list(range(N_CORES))],
                    ins=[cc_in.opt()],
                    outs=[cc_out.opt()],
                )
                sg_s = wp.tile([128, N_CORES * 2 * UO], F32, tag="sg")
                nc.sync.dma_start(
                    sg_s[:], cc_out.opt().rearrange("(r g p) f -> p (r g) f", r=N_CORES, g=2)
                )
                t1 = wp.tile([128, 4 * 2 * UO], F32, tag="t1")
                nc.vector.tensor_add(t1[:], sg_s[:, :4 * 2 * UO], sg_s[:, 4 * 2 * UO:])
                t2 = wp.tile([128, 2 * 2 * UO], F32, tag="t2")
                nc.vector.tensor_add(t2[:], t1[:, :2 * 2 * UO], t1[:, 2 * 2 * UO:])
                s_sb = wp.tile([128, 2 * UO], F32, tag="s_sb")
                nc.vector.tensor_add(s_sb[:], t2[:, :2 * UO], t2[:, 2 * UO:])

                # ---- squash: v = s * sqrt(n2) / (1 + n2) ----
                sq = wp.tile([128, 2 * UO], F32, tag="sq")
                nc.vector.tensor_mul(sq[:], s_sb[:], s_sb[:])
                n2 = wp.tile([128, 2 * U], F32, tag="n2")
                nc.vector.reduce_sum(
                    n2[:], sq[:].rearrange("p (t u o) -> p (t u) o", t=2, u=U),
                    axis=mybir.AxisListType.X,
                )
                rt = wp.tile([128, 2 * U], F32, tag="rt")
                nc.scalar.sqrt(rt[:], n2[:])
                if not last:
                    # prewarm the Exp ACT table while G/sel run (dep on rt orders it)
                    nc.scalar.activation(
                        scr[:, 1:2], rt[:, 0:1], mybir.ActivationFunctionType.Exp
                    )
                dn = wp.tile([128, 2 * U], F32, tag="dn")
                nc.vector.tensor_scalar_add(dn[:], n2[:], 1.0)
                rd = wp.tile([128, 2 * U], F32, tag="rd")
                nc.vector.reciprocal(rd[:], dn[:])
                f = wp.tile([128, 2 * U], F32, tag="f")
                nc.vector.tensor_mul(f[:], rt[:], rd[:])
                if last:
                    v_out = v_last[:].rearrange("p (t u o) -> p t u o", t=2, u=U)
                else:
                    v_out = v_s[:].rearrange("p (t q o) -> p t q o", t=2, q=16)[:, :, :U, :].bitcast(F32R)
                nc.vector.tensor_mul(
                    v_out,
                    s_sb[:].rearrange("p (t u o) -> p t u o", t=2, u=U),
                    f[:].rearrange("p (t u) -> p t u", t=2).unsqueeze(3).broadcast_to((128, 2, U, O)),
                )

                if last:
                    nc.sync.dma_start(
                        out_d.rearrange("(g p) u o one -> p g (u o one)", g=2),
                        v_last[:].rearrange("p (t f) -> p t f", t=2),
                    )
                    break

                # ---- G[(c,i),(u,o)] = sum_b x*v ; pm2 = sum_o G*W ; agr = sel^T pm2 ----
                for m in range(K):
                    g_ps = gps.tile([128, UOP], F32, tag="g_ps")
                    for t in range(2):
                        _mm(
                            nc, g_ps[:],
                            xb_s[:, t * CI + m * 128: t * CI + (m + 1) * 128],
                            v_s[:, t * UOP:(t + 1) * UOP],
                            start=(t == 0), stop=(t == 1), fast=True,
                        )
                    pm = wp.tile([128, UO], F32, tag="pm")
                    nc.vector.tensor_mul(pm[:], g_ps[:, :UO], w1_s[:, m * UOP: m * UOP + UO])
                    nc.vector.reduce_sum(
                        pm2_s[:, m * U:(m + 1) * U],
                        pm[:].rearrange("p (u o) -> p u o", u=U),
                        axis=mybir.AxisListType.X,
                    )
                a_ps = aps.tile([16, K * U], F32, tag="a_ps")
                nc.tensor.matmul(a_ps[:], lhsT=sel_s[:], rhs=pm2_s[:], start=True, stop=True)

                # ---- logits update + softmax over u (c-local, tiny) ----
                if r == 0:
                    nc.vector.tensor_copy(b_state[:], a_ps[:])
                else:
                    nc.vector.tensor_add(b_state[:], b_state[:], a_ps[:])
                eb = wp.tile([16, K * U], F32, tag="eb")
                nc.scalar.activation(eb[:], b_state[:], mybir.ActivationFunctionType.Exp)
                # prewarm the Sqrt ACT table for the next squash
                nc.scalar.activation(
                    scr[:16, 0:1], eb[:, 0:1], mybir.ActivationFunctionType.Sqrt
                )
                den = wp.tile([16, K], F32, tag="den")
                nc.vector.reduce_sum(
                    den[:], eb[:].rearrange("p (k u) -> p k u", k=K),
                    axis=mybir.AxisListType.X,
                )
                rden = wp.tile([16, K], F32, tag="rden")
                nc.vector.reciprocal(rden[:], den[:])
                cnorm = wp.tile([16, K * U], F32, tag="cnorm")
                nc.vector.tensor_mul(
                    cnorm[:].rearrange("p (k u) -> p k u", k=K),
                    eb[:].rearrange("p (k u) -> p k u", k=K),
                    rden[:].unsqueeze(2).broadcast_to((16, K, U)),
                )

                # ---- expand coef to (c,i) partitions; W_eff = W * coef ----
                cx_ps = cxps.tile([128, K * U], F32, tag="cx")
                nc.tensor.matmul(cx_ps[:], lhsT=exp_s[:], rhs=cnorm[:], start=True, stop=True)
                cx_sb = wp.tile([128, K * U], F32, tag="cx_sb")
                nc.vector.tensor_copy(cx_sb[:], cx_ps[:])
                if r < NROUTE - 2 or FAST_LAST_S:
                    weff_out = weff_s[:].rearrange("p (k q o) -> p k q o", k=K, q=16)[:, :, :U, :].bitcast(F32R)
                    weff = weff_s
                else:
                    weff_out = weff_last[:].rearrange("p (k u o) -> p k u o", k=K, u=U)
                    weff = weff_last
                nc.vector.tensor_mul(
                    weff_out,
                    w1_s[:].rearrange("p (k q o) -> p k q o", k=K, q=16)[:, :, :U, :],
                    cx_ps[:].rearrange("p (k u) -> p k u", k=K).unsqueeze(3).broadcast_to((128, K, U, O)),
                )

    nc.compile()
    return nc


_PROGRAM_CACHE = {}


def _get_program():
    if "nc" not in _PROGRAM_CACHE:
        _PROGRAM_CACHE["nc"] = _build_program()
    return _PROGRAM_CACHE["nc"]


def _make_in_maps(x, W):
    x = np.ascontiguousarray(x, dtype=np.float32)
    W = np.ascontiguousarray(W, dtype=np.float32)
    sel = np.zeros((128, 16), dtype=np.float32)
    for p in range(128):
        sel[p, p // IU] = 1.0 / B
    exp16 = np.zeros((16, 128), dtype=np.float32)
    for p in range(128):
        exp16[p // IU, p] = 10.0  # cancels the 0.1 pre-scale of xp

    in_maps = []
    for core in range(N_CORES):
        c0 = core * CL
        xc = x[:, :, c0:c0 + CL]                    # [B, I, CL]
        Wc = W[c0:c0 + CL]                          # [CL, U, O, I]
        # xp[p, k*B + b] = 0.1 * x[b, i, c], ci = k*128+p = c_rel*8+i
        xp = 0.1 * xc.transpose(2, 1, 0).reshape(CI, B)
        xp = np.ascontiguousarray(
            xp.reshape(K, 128, B).transpose(1, 0, 2).reshape(128, K * B)
        )
        # xb[p, t*CI + ci] = x[t*128+p, i, c]
        xb = xc.transpose(0, 2, 1).reshape(B, CI)
        xb = np.ascontiguousarray(
            xb.reshape(2, 128, CI).transpose(1, 0, 2).reshape(128, 2 * CI)
        )
        # w1[p, k*UOP + uo] = W[c, u, o, i], zero-padded to UOP per k-tile
        w1 = Wc.transpose(0, 3, 1, 2).reshape(CI, UO).reshape(K, 128, UO)
        w1p = np.zeros((128, K, UOP), dtype=np.float32)
        w1p[:, :, :UO] = w1.transpose(1, 0, 2)
        w1p = np.ascontiguousarray(w1p.reshape(128, K * UOP))
        zc = np.zeros((128, K * (UOP - UO)), dtype=np.float32)
        in_maps.append(
            {"xp": xp, "xb": xb, "w1": w1p, "sel": sel, "exp16": exp16, "zc": zc}
        )
    return in_maps


def kernel(x, W, _trace=False, _trace_kwargs=None):
    nc = _get_program()
    in_maps = _make_in_maps(x, W)
    res = run_bass_kernel_spmd(
        nc, in_maps, core_ids=list(range(N_CORES)), trace=_trace,
        **(_trace_kwargs or {}),
    )
    out = res.results[0]["out"].astype(np.float32).reshape(B, U, O, 1)
    if _trace:
        kernel.last_results = res
    return out
